# revision 5
# baseline (speedup 1.0000x reference)
"""DIN attention v5: v3 + count-sorted rows with per-group batch shrink.

Rows are assigned to cores round-robin (so all cores see the same count
distribution) and sorted within each core by descending unmasked count.
Slot-group g (slots 4g..4g+3) then only needs the leading rows whose count
exceeds 4g; the number of live 128-row b-blocks per group (the "profile")
shrinks from 4 down to 1 over the tail groups, cutting activation / matmul /
DMA work by the padding tax (~13% for Bernoulli(0.5) masks). Logit columns
of skipped blocks are never written; copy_predicated overwrites them with
NEG_BIG (the rows there are guaranteed pad), so softmax semantics are
unchanged.

v3 recap below:

Per group of 4 slots, a "feat" mega-tile [96, 4, 512] bf16 holds the three
K=32 operand strips stacked along partitions: rows 0-31 = qp (static,
written once in setup into both double-buffered feat tiles), rows 32-63 = k
(DMA), rows 64-95 = qp*k (second k DMA, then in-place DVE multiply against a
static qp replica at the same partition base). Layer 1 is then ONE K=96
matmul per slot with a single resident stationary w196 [96, 80] -- a quarter
of v2's L1 matmul+ldweights instruction count.

L2 (col-tiled pairs into [128, 512] banks), L3 (stationary a2 chunk [128,
128] x rhs wfpad [128, 2] -> b-major logits), and the b-major masked softmax
are as in v2.
"""

import contextlib

import numpy as np

import concourse.bacc as bacc
import concourse.mybir as mybir
import concourse.tile as tile
from concourse.bass_utils import run_bass_kernel_spmd

N_CORES = 8
B, T, D = 4096, 200, 32
BC = B // N_CORES  # 512 rows per core
H1, H2 = 80, 40
NBLK = BC // 128   # 4 blocks of 128 b's
NEG_BIG = float(np.float32(-2.0**32 + 1.0))

F32 = mybir.dt.float32
F32R = mybir.dt.float32r
BF16 = mybir.dt.bfloat16
I8 = mybir.dt.int8
AF = mybir.ActivationFunctionType
ALU = mybir.AluOpType
AX = mybir.AxisListType


def _emit(nc, tc, es, d, TT, profile, repeat=1):
    const = es.enter_context(tc.tile_pool(name="const", bufs=1))
    a1p = es.enter_context(tc.tile_pool(name="a1p", bufs=4))
    a2p = es.enter_context(tc.tile_pool(name="a2p", bufs=4))
    ps1p = es.enter_context(tc.tile_pool(name="ps1p", bufs=2, space="PSUM"))
    ps2p = es.enter_context(tc.tile_pool(name="ps2p", bufs=1, space="PSUM"))
    psLp = es.enter_context(tc.tile_pool(name="psLp", bufs=1, space="PSUM"))

    # ---- static tiles ----
    w196 = const.tile([96, H1], BF16)        # [W1q+W1m; W1k-W1m; W1p] stacked
    w2pad = const.tile([H1, 64], BF16)
    wfpad = const.tile([128, 2], BF16)
    wq = const.tile([D, D], F32R)
    b1s = const.tile([H1, 1], F32)
    b2pad = const.tile([128, 1], F32)
    bqs = const.tile([D, 1], F32)
    als = const.tile([D, 1], F32)
    qts = const.tile([D, BC], F32R)
    qp1 = const.tile([D, BC], BF16)          # qp^T bf16 (single strip)
    feats = [const.tile([96, 4, BC], BF16, name=f"feat{i}")
             for i in range(6)]               # [qp | k | qp*k] x 4 slots
    qpstat = const.tile([96, 4, BC], BF16)   # rows 64-95: qp replica per slot
    mki = const.tile([128, NBLK, TT], I8)
    negb = const.tile([128, NBLK * TT], F32)
    tmpr = const.tile([D, BC], F32)
    tmpa = const.tile([D, BC], F32)
    tmpb = const.tile([D, BC], F32)
    mx = const.tile([128, NBLK], F32)
    sums = const.tile([128, NBLK], F32)
    rin = const.tile([128, NBLK], F32)
    expv = const.tile([128, NBLK, TT], F32)
    att = const.tile([128, NBLK, TT], F32)

    nc.sync.dma_start(out=w196, in_=d["w196"])
    nc.sync.dma_start(out=w2pad, in_=d["w2pad"])
    nc.sync.dma_start(out=wfpad, in_=d["wfpad"])
    nc.sync.dma_start(out=wq, in_=d["Wq"])
    nc.sync.dma_start(out=b1s, in_=d["b1"])
    nc.sync.dma_start(out=b2pad, in_=d["b2pad"])
    nc.sync.dma_start(out=bqs, in_=d["bq"])
    nc.sync.dma_start(out=als, in_=d["alpha"])
    nc.sync.dma_start(out=qts, in_=d["qT"])
    nc.sync.dma_start(out=mki, in_=d["mki"])

    # dummy sigmoid: hoist the activation-table load into setup
    nc.scalar.activation(tmpr[:, 0:1], bqs, AF.Sigmoid)

    # qp^T = prelu(Wq^T @ q^T + bq, alpha), cast to bf16
    ps0 = ps1p.tile([D, BC], F32, tag="ps1")
    nc.tensor.matmul(ps0, wq, qts, start=True, stop=True)
    nc.vector.tensor_scalar(tmpr, ps0, bqs, 0.0, op0=ALU.add, op1=ALU.max)
    nc.vector.tensor_scalar(tmpa, ps0, bqs, 0.0, op0=ALU.add, op1=ALU.min)
    nc.vector.tensor_scalar(tmpb, tmpa, als, None, op0=ALU.mult)
    nc.vector.tensor_add(qp1, tmpr, tmpb)
    # replicate qp into the static strips: feat rows 0-31 (both buffers) and
    # qpstat rows 64-95, for each of the 4 slot positions
    for s in range(4):
        for f in feats:
            nc.sync.dma_start(out=f[0:32, s, :], in_=qp1)
        nc.sync.dma_start(out=qpstat[64:96, s, :], in_=qp1)

    nc.vector.memset(negb, NEG_BIG)

    logAB = psLp.tile([128, NBLK, TT], F32)  # one bank: 4*124 = 496 f32

    for _rep in range(repeat):
        _main_pass(nc, d, TT, profile, a1p, a2p, ps1p, ps2p, w196, w2pad,
                   wfpad, b1s, b2pad, feats, qpstat, mki, negb, mx,
                   sums, rin, expv, att, logAB)


def _main_pass(nc, d, TT, profile, a1p, a2p, ps1p, ps2p, w196, w2pad, wfpad,
               b1s, b2pad, feats, qpstat, mki, negb, mx, sums, rin,
               expv, att, logAB):
    for g in range(TT // 4):
        n = 128 * profile[g]          # live rows of this slot group
        f = feats[g % 6]
        nc.sync.dma_start(out=f[32:64, :, 0:n], in_=d["kP"][g][:, :, 0:n])
        nc.gpsimd.dma_start(out=f[64:96, :, 0:n], in_=d["kP"][g][:, :, 0:n])
        ps2 = ps2p.tile([128, 2, BC], F32)
        for pair in range(2):
            ps1 = ps1p.tile([H1, 2, BC], F32, tag="ps1")
            for i in range(2):
                j = 2 * pair + i
                sj = slice(j, j + 1)
                nc.vector.tensor_mul(f[64:96, sj, 0:n], f[64:96, sj, 0:n],
                                     qpstat[64:96, sj, 0:n])
                nc.tensor.matmul(ps1[:, i, 0:n], w196, f[:, j, 0:n],
                                 start=True, stop=True)
            a1 = a1p.tile([H1, 2, BC], BF16)
            nc.scalar.activation(a1[:, :, 0:n], ps1[:, :, 0:n], AF.Sigmoid,
                                 bias=b1s)
            # L2: two slots of the pair at col offsets 0 / 64 into one bank
            nc.tensor.matmul(ps2[0:64, pair, 0:n], w2pad, a1[:, 0, 0:n],
                             start=True, stop=True, tile_position=(0, 0))
            nc.tensor.matmul(ps2[64:128, pair, 0:n], w2pad, a1[:, 1, 0:n],
                             start=True, stop=True, tile_position=(0, 64))
        a2 = a2p.tile([128, 2, BC], BF16)
        nc.scalar.activation(a2[:, :, 0:n], ps2[:, :, 0:n], AF.Sigmoid,
                             bias=b2pad)
        for pair in range(2):
            t0 = 4 * g + 2 * pair
            for jj in range(profile[g]):
                nc.tensor.matmul(
                    logAB[:, jj, t0:t0 + 2],
                    a2[:, pair, 128 * jj:128 * jj + 128],
                    wfpad,
                    start=True,
                    stop=True,
                )

    # ---- masked softmax over t ----
    nc.vector.copy_predicated(logAB, mki, negb.rearrange("p (g t) -> p g t", g=NBLK))
    nc.vector.tensor_reduce(mx, logAB, axis=AX.X, op=ALU.max, negate=True)
    for blk in range(NBLK):
        nc.scalar.activation(
            expv[:, blk, :],
            logAB[:, blk, :],
            AF.Exp,
            bias=mx[:, blk:blk + 1],
            accum_out=sums[:, blk:blk + 1],
        )
    nc.vector.reciprocal(rin, sums)
    for blk in range(NBLK):
        nc.vector.tensor_scalar(
            att[:, blk, :], expv[:, blk, :], rin[:, blk:blk + 1], None,
            op0=ALU.mult,
        )
    nc.sync.dma_start(
        out=d["out"].rearrange("(blk p) t -> p blk t", blk=NBLK), in_=att
    )


def build(TT=T, profile=None, repeat=1):
    if profile is None:
        profile = (NBLK,) * (TT // 4)
    nc = bacc.Bacc("TRN2", target_bir_lowering=False, debug=False,
                   num_devices=N_CORES)
    d = {
        "kP": nc.dram_tensor("kP", [TT // 4, D, 4, BC], BF16, kind="ExternalInput").ap(),
        "qT": nc.dram_tensor("qT", [D, BC], F32R, kind="ExternalInput").ap(),
        "mki": nc.dram_tensor("mki", [128, NBLK, TT], I8, kind="ExternalInput").ap(),
        "Wq": nc.dram_tensor("Wq", [D, D], F32R, kind="ExternalInput").ap(),
        "bq": nc.dram_tensor("bq", [D, 1], F32, kind="ExternalInput").ap(),
        "alpha": nc.dram_tensor("alpha", [D, 1], F32, kind="ExternalInput").ap(),
        "w196": nc.dram_tensor("w196", [96, H1], BF16, kind="ExternalInput").ap(),
        "b1": nc.dram_tensor("b1", [H1, 1], F32, kind="ExternalInput").ap(),
        "w2pad": nc.dram_tensor("w2pad", [H1, 64], BF16, kind="ExternalInput").ap(),
        "b2pad": nc.dram_tensor("b2pad", [128, 1], F32, kind="ExternalInput").ap(),
        "wfpad": nc.dram_tensor("wfpad", [128, 2], BF16, kind="ExternalInput").ap(),
        "out": nc.dram_tensor("out", [BC, TT], F32, kind="ExternalOutput").ap(),
    }
    with tile.TileContext(nc) as tc:
        with contextlib.ExitStack() as es:
            _emit(nc, tc, es, d, TT, profile, repeat=repeat)
    nc.compile()
    return nc


def prepare(q, k, mask, Wq, bq, alpha, W1, b1, W2, b2, Wf):
    """Varlen packing + host-side weight prep (combine/pad/cast to bf16)."""
    import ml_dtypes
    bf16 = ml_dtypes.bfloat16

    mask_np = np.asarray(mask)
    cnt = (mask_np != 0).sum(1)
    if cnt.min() == 0:
        # fully-masked row: identity "compaction" (T is a multiple of 4)
        TT = T
        tidx = np.ascontiguousarray(np.tile(np.arange(T), (B, 1)))
        pad = mask_np == 0
    else:
        TT = int(-(-int(cnt.max()) // 4) * 4)
        TT = max(TT, 8)
        order = np.argsort(mask_np == 0, axis=1, kind="stable")
        tidx = np.ascontiguousarray(order[:, :TT])
        pad = (np.arange(TT)[None, :] >= cnt[:, None])
    kc = np.take_along_axis(np.asarray(k), tidx[:, :, None], axis=1)

    # round-robin rows to cores, sort each core's rows by descending count;
    # rowmap[c, i] = original batch row at core c, sorted position i
    rowmap = np.empty((N_CORES, BC), np.int64)
    for c in range(N_CORES):
        rows = np.arange(c, B, N_CORES)
        rowmap[c] = rows[np.argsort(-cnt[rows], kind="stable")]
    # per-group live-row profile: group g needs rows with cnt > 4g, rounded
    # up to whole 128-row blocks; take the max over cores
    profile = []
    scnt = np.sort(cnt.reshape(-1, N_CORES).T, axis=1)[:, ::-1]  # striped+sorted
    for g in range(TT // 4):
        need = int((scnt > 4 * g).sum(1).max())
        # floor 2 blocks: n=128 would give 256B DMA descriptors (< 512B
        # SDMA line-rate threshold)
        profile.append(min(NBLK, max(2, -(-need // 128))))
    profile = tuple(profile)

    W1q, W1k, W1m, W1p = (np.asarray(W1, np.float64)[i * D:(i + 1) * D]
                          for i in range(4))
    w196 = np.concatenate([W1q + W1m, W1k - W1m, W1p], axis=0).astype(bf16)  # [96, 80]
    w2pad = np.zeros((H1, 64), bf16)
    w2pad[:, :H2] = np.asarray(W2, np.float32).astype(bf16)
    wfpad = np.zeros((128, 2), bf16)
    wfv = np.asarray(Wf, np.float32).reshape(H2).astype(bf16)
    wfpad[0:H2, 0] = wfv
    wfpad[64:64 + H2, 1] = wfv
    b2pad = np.zeros((128, 1), np.float32)
    b2pad[0:H2, 0] = np.asarray(b2, np.float32)
    b2pad[64:64 + H2, 0] = np.asarray(b2, np.float32)

    common = {
        "Wq": np.ascontiguousarray(Wq, np.float32),
        "bq": np.ascontiguousarray(bq, np.float32).reshape(D, 1),
        "alpha": np.ascontiguousarray(alpha, np.float32).reshape(D, 1),
        "w196": w196,
        "b1": np.ascontiguousarray(b1, np.float32).reshape(H1, 1),
        "w2pad": w2pad,
        "b2pad": b2pad,
        "wfpad": wfpad,
    }
    in_maps = []
    for c in range(N_CORES):
        sl = rowmap[c]
        # [TT, D, BC] -> [TT//4, D, 4, BC]: per group, partition d rows hold
        # the 4 slots' k side by side along the free dim
        kt = kc[sl].transpose(1, 2, 0)                   # [TT, D, BC]
        kp = np.ascontiguousarray(
            kt.reshape(TT // 4, 4, D, BC).transpose(0, 2, 1, 3).astype(bf16))
        qc = np.ascontiguousarray(np.asarray(q)[sl].T, np.float32)  # [D, BC]
        mc = pad[sl].astype(np.int8)
        mc = np.ascontiguousarray(mc.reshape(NBLK, 128, TT).transpose(1, 0, 2))
        m = dict(common)
        m.update({"kP": kp, "qT": qc, "mki": mc})
        in_maps.append(m)
    return in_maps, TT, tidx, rowmap, profile


def postprocess(results, TT, tidx, rowmap):
    attc = np.empty((B, TT), np.float32)
    for c in range(N_CORES):
        attc[rowmap[c]] = results[c]["out"]
    out = np.zeros((B, T), np.float32)
    np.put_along_axis(out, tidx, attc, axis=1)
    return out.reshape(B, 1, T)


_NC_CACHE = {}


def kernel(**inputs):
    in_maps, TT, tidx, rowmap, profile = prepare(
        inputs["q"], inputs["k"], inputs["mask"], inputs["Wq"], inputs["bq"],
        inputs["alpha"], inputs["W1"], inputs["b1"], inputs["W2"], inputs["b2"],
        inputs["Wf"],
    )
    key = (TT, profile)
    if key not in _NC_CACHE:
        _NC_CACHE[key] = build(TT=TT, profile=profile)
    nc = _NC_CACHE[key]
    res = run_bass_kernel_spmd(nc, in_maps, core_ids=list(range(N_CORES)))
    return postprocess(res.results, TT, tidx, rowmap)
